# revision 70
# baseline (speedup 1.0000x reference)
"""ExpandingLinear (sparse EmbedLinear + sparse ExpandingLinear tail) on 8 trn2 cores.

Math:
    h  = relu(x @ W_e.T)          W_e sparse [R_EMB, F_IN]  (COO, 6.25% dense)
    x2 = concat([x, h], axis=1)
    y  = x2 @ W.T + bias          W   sparse [F_OUT, F_MID], bias sparse [F_OUT]

Strategy: densify the sparse weights on the host (one-time weight prep,
O(nnz) work), then run the O(nnz * B) compute as two dense matmuls on the
TensorEngine.  Data-parallel over the batch: each of the 8 cores gets
B/8 = 256 rows of x (as x.T columns) and the full dense weights.

Device schedule (per core):
    xt_sb  [128, 8*256]      x.T tiles resident in SBUF (f-major)
    we/wt  [128, G*1024] x k W_e.T / W.T row-block stripes streamed on the
                             sync HWDGE ring; PE chases the arrival front
    warm-up: ~3.4us of garbage matmuls so the HAM clock gate is at 2.4 GHz
    MM1: psum_h[r] += we[f,r].T @ xt[f]    -> relu -> hT[r] [128, 256]
    MM2: psum[b,oc] = ones.T @ bias_row    (K=1 bias broadcast, first)
                    += x2t[c,b].T @ wt[c,oc]  N=512, x2t = xt ++ hT tiles
    out: psum -> SBUF copies on DVE/ACT in parallel -> natural-layout
         [256, 1024] stores split across both HWDGE rings

Post-passes work around / trim framework overheads: _split_excess_waits
(walrus here caps sync waits at 1/instruction), _hoist_preamble_work
(start the DMA pipe during the preamble), and a leaner TileContext tail.

Modes (KERNEL_MODE env): "bf16" (default, ~42us, rel err ~1e-3) or "f32r"
(fp32 storage, full-rate fp32r matmuls, ~52us, rel err ~1.4e-4).
"""

import os

import numpy as np

B = 2048
F_IN = 1024
R_EMB = 1024
F_OUT = 1024
F_MID = F_IN + R_EMB
N_CORES = 8
B_SH = B // N_CORES  # 256

P = 128
NF = F_IN // P    # 8 f-tiles (MM1 contraction)
NR = R_EMB // P   # 8 r-tiles (MM1 outputs / psum tiles)
NC_T = F_MID // P  # 16 c-tiles (MM2 contraction)
NO = F_OUT // P   # 8 o-tiles (MM2 outputs)

_cache = {}


def _split_excess_waits(nc, mybir, max_waits=1):
    """Walrus in this container rejects instructions with >1 sync waits
    ("Too many sync wait commands").  Hoist excess waits onto same-engine
    NOPs placed immediately before the offending instruction."""
    cnt = 0
    for f in nc.m.functions:
        for b in f.blocks:
            out = []
            for inst in b.instructions:
                si = inst.sync_info
                if si is not None and len(si.on_wait) > max_waits:
                    waits = list(si.on_wait)
                    keep = waits[-max_waits:]
                    hoist = waits[:-max_waits]
                    for j in range(0, len(hoist), max_waits):
                        chunk = hoist[j : j + max_waits]
                        out.append(
                            mybir.InstNoOp(
                                name=f"{inst.name}_splitw{j}",
                                engine=inst.engine,
                                sync_info=mybir.SyncInfo(on_wait=chunk, on_update=[]),
                                bass_nofuse=True,
                            )
                        )
                        cnt += 1
                    inst.sync_info = mybir.SyncInfo(
                        on_wait=keep, on_update=list(si.on_update)
                    )
                out.append(inst)
            b.instructions = out
    return cnt


def _hoist_preamble_work(nc, mybir, max_dmas=2):
    """Move the first `max_dmas` wait-free SP load DMAs and the first
    wait-free Pool memset from the tile block into the main block, ahead
    of each engine's start-barrier EVSEM.  They only touch
    freshly-allocated SBUF, so running them during the preamble is safe;
    the DMA pipe starts ~3us earlier and the PE warm-up source is ready
    as soon as PE clears the barrier."""
    f = nc.m.functions[0]
    b0, b1 = f.blocks[0], f.blocks[1]
    moved_sp, moved_pool, rest = [], [], []
    sp_prefix = pool_prefix = True
    moved_pe = []
    pe_prefix = True
    for inst in b1.instructions:
        nm = type(inst).__name__
        si = inst.sync_info
        waits = bool(si and si.on_wait)
        if sp_prefix and inst.engine == mybir.EngineType.SP:
            if "DMA" in nm and not waits and len(moved_sp) < max_dmas:
                moved_sp.append(inst)
                continue
            sp_prefix = False
        if pool_prefix and inst.engine == mybir.EngineType.Pool:
            if nm == "InstMemset" and not waits:
                moved_pool.append(inst)
                pool_prefix = False
                continue
            pool_prefix = False
        if pe_prefix and inst.engine == mybir.EngineType.PE:
            # warm-up matmuls (they reference the wsrc tile): run them
            # during the start barrier so HAM is warm when real work begins;
            # their memset wait travels with them (memset is hoisted too)
            if nm in ("InstMatmult", "InstLdweights") and "wsrc" in inst.pretty_str():
                moved_pe.append(inst)
                continue
            pe_prefix = False
        rest.append(inst)
    il0 = list(b0.instructions)
    sp_pos = next(
        (i for i, inst in enumerate(il0) if inst.engine == mybir.EngineType.SP),
        len(il0),
    )
    il0 = il0[:sp_pos] + moved_sp + il0[sp_pos:]
    pool_pos = next(
        (
            i
            for i, inst in enumerate(il0)
            if inst.engine == mybir.EngineType.Pool
            and type(inst).__name__ == "InstDrain"
        ),
        len(il0),
    )
    il0 = il0[:pool_pos] + moved_pool + il0[pool_pos:]
    pe_pos = next(
        (
            i
            for i, inst in enumerate(il0)
            if inst.engine == mybir.EngineType.PE
            and type(inst).__name__ == "InstDrain"
        ),
        len(il0),
    )
    il0 = il0[:pe_pos] + moved_pe + il0[pe_pos:]
    b0.instructions = il0
    b1.instructions = rest
    return len(moved_sp) + len(moved_pool) + len(moved_pe)


def _build(mode):
    import concourse.bass as bass
    import concourse.mybir as mybir
    import concourse.tile as tile

    # Leaner kernel tail: the stock _drain_and_barrier runs
    # drain -> barrier -> sem clears -> barrier.  The final barrier only
    # makes the other engines wait for SP's sem clears; execution ends when
    # every engine stream ends either way, so drop it.
    if not getattr(tile.TileContext, "_lean_tail", False):
        def _drain_and_barrier(self, tick_clock, wait_clock):
            from concourse.vector_clock import ScopedClock

            drain_inst = self.nc.sync.drain()
            wait_clock.add_sem_waits(
                drain_inst.ins, ScopedClock({None: tick_clock.global_clock})
            )
            self.nc.all_engine_barrier()
            assert self.sems is not None
            popped = self.nc._tile_sem_poison_stack.pop()
            assert popped is self._sem_poison
            self.nc.clear_and_free_semaphores(list(self.sems.allocated().values()))

        tile.TileContext._drain_and_barrier = _drain_and_barrier
        tile.TileContext._lean_tail = True

    dt = mybir.dt
    mm_dt = dt.float32r if mode == "f32r" else dt.bfloat16
    G1 = 1  # row-blocks per MM1 weight stripe (small: earliest PE start)
    G2 = 2  # row-blocks per MM2 weight stripe

    nc = bass.Bass("TRN2", target_bir_lowering=False, debug=False, num_devices=N_CORES)

    f32r = dt.float32r
    xT = nc.declare_dram_parameter("xT", [F_IN, B_SH], mm_dt, isOutput=False)
    weT = nc.declare_dram_parameter("weT", [F_IN, R_EMB], mm_dt, isOutput=False)
    wT = nc.declare_dram_parameter("wT", [F_MID, F_OUT], mm_dt, isOutput=False)
    bias_row = nc.declare_dram_parameter("bias_row", [1, F_OUT], f32r, isOutput=False)
    ones_row = nc.declare_dram_parameter("ones_row", [1, P], f32r, isOutput=False)
    outN = nc.declare_dram_parameter("outN", [B_SH, F_OUT], dt.float32, isOutput=True)

    with tile.TileContext(nc) as tc:
        with (
            tc.tile_pool(name="xt", bufs=1) as xt_pool,
            tc.tile_pool(name="w", bufs=8) as w_pool,
            tc.tile_pool(name="h", bufs=NR) as h_pool,
            tc.tile_pool(name="ot", bufs=NO) as out_pool,
            tc.tile_pool(name="bias", bufs=1) as bias_pool,
            tc.tile_pool(name="psum", bufs=8, space="PSUM") as psum_pool,
        ):
            # PE warm-up source: memset tile (always bf16 — warm-up only
            # exists to keep the HAM clock busy; values are irrelevant)
            wsrc = bias_pool.tile([P, B_SH], dt.bfloat16, name="wsrc")
            nc.gpsimd.memset(wsrc[:], 0)
            # dummy ACT op: pulls the activation LUT load into the preamble
            # window (internal queue) instead of lazily on the output path
            act_warm = bias_pool.tile([P, 2], dt.float32, name="act_warm")
            nc.gpsimd.memset(act_warm[:], 0)
            nc.scalar.activation(
                act_warm[:, 0:1], act_warm[:, 1:2],
                mybir.ActivationFunctionType.Identity,
            )

            # x.T first on the sync ring (gates all of MM1); bias on scalar
            xt_sb = xt_pool.tile([P, NF * B_SH], mm_dt)
            nc.sync.dma_start(
                out=xt_sb[:].rearrange("p (f b) -> p f b", b=B_SH),
                in_=xT[:].rearrange("(f p) b -> p f b", p=P),
            )
            bias_sb = bias_pool.tile([1, F_OUT], f32r, name="bias_sb")
            nc.scalar.dma_start(out=bias_sb[:], in_=bias_row[:])
            ones_sb = bias_pool.tile([1, P], f32r, name="ones_sb")
            nc.scalar.dma_start(out=ones_sb[:], in_=ones_row[:])

            # weight stripes on the sync ring only (a single HWDGE ring
            # sustains ~320 GB/s; splitting across two measured slower)
            we_t = []
            for k in range(NF // G1):
                t = w_pool.tile([P, G1 * R_EMB], mm_dt, tag="we", name=f"we{k}")
                nc.sync.dma_start(
                    out=t[:].rearrange("p (f r) -> p f r", r=R_EMB),
                    in_=weT[G1 * k * P : G1 * (k + 1) * P, :].rearrange(
                        "(f p) r -> p f r", p=P
                    ),
                )
                we_t.append(t)
            wt_t = []
            for k in range(NC_T // G2):
                t = w_pool.tile([P, G2 * F_OUT], mm_dt, tag="wt", name=f"wt{k}")
                nc.sync.dma_start(
                    out=t[:].rearrange("p (c o) -> p c o", o=F_OUT),
                    in_=wT[G2 * k * P : G2 * (k + 1) * P, :].rearrange(
                        "(c p) o -> p c o", p=P
                    ),
                )
                wt_t.append(t)

            # MM1: psum_h[r] = sum_f we[f, r-block].T @ xt[f]
            psum_h = [
                psum_pool.tile([P, B_SH], dt.float32, tag="acc", name=f"ph{r}")
                for r in range(NR)
            ]

            # PE warm-up: ~4us of garbage matmuls starting right after the
            # memset so the HAM clock gate is at 2.4 GHz when real matmuls
            # start.  Results land in psum_h[0] partitions 0-7 and are wiped
            # by MM1's start=True.
            for _ in range(16):
                nc.tensor.matmul(
                    out=psum_h[0][0:8, :],
                    lhsT=wsrc[:, 0:8],
                    rhs=wsrc[:],
                    start=True,
                    stop=True,
                )
            for f in range(NF):
                lhs_base = we_t[f // G1]
                off = (f % G1) * R_EMB
                rhs = xt_sb[:, f * B_SH : (f + 1) * B_SH]
                for r in range(NR):
                    nc.tensor.matmul(
                        out=psum_h[r][:],
                        lhsT=lhs_base[:, off + r * P : off + (r + 1) * P],
                        rhs=rhs,
                        start=(f == 0),
                        stop=(f == NF - 1),
                    )

            hT = []
            for r in range(NR):
                t = h_pool.tile([P, B_SH], mm_dt, tag="h", name=f"h{r}")
                nc.vector.tensor_scalar_max(t[:], psum_h[r][:], 0.0)
                hT.append(t)

            # MM2 (activations stationary, weights moving, N=512):
            #   out[b-block, o-chunk] = sum_c x2t[c, b-block].T @ wt[c, o-chunk]
            # x2t tiles are the resident xt slices (c<8) and hT tiles (c>=8).
            NB = 2   # b-blocks of 128
            NOC = 2  # o-chunks of 512
            OC = F_OUT // NOC
            psum_b = [
                psum_pool.tile([P, OC], dt.float32, tag="acc", name=f"pb{i}")
                for i in range(NB * NOC)
            ]
            # bias first via K=1 matmul: psum = ones.T @ bias_row (broadcast
            # over b); keeps the bias add off the kernel tail
            for bb in range(NB):
                for oc in range(NOC):
                    nc.tensor.matmul(
                        out=psum_b[bb * NOC + oc][:],
                        lhsT=ones_sb[:],
                        rhs=bias_sb[:, oc * OC : (oc + 1) * OC],
                        start=True,
                        stop=False,
                    )
            for c in range(NC_T):
                rhs_base = wt_t[c // G2]
                off = (c % G2) * F_OUT
                for bb in range(NB):
                    if c < NF:
                        lhsT = xt_sb[:, c * B_SH + bb * P : c * B_SH + (bb + 1) * P]
                    else:
                        lhsT = hT[c - NF][:, bb * P : (bb + 1) * P]
                    for oc in range(NOC):
                        nc.tensor.matmul(
                            out=psum_b[bb * NOC + oc][:],
                            lhsT=lhsT,
                            rhs=rhs_base[:, off + oc * OC : off + (oc + 1) * OC],
                            start=False,
                            stop=(c == NC_T - 1),
                        )

            for bb in range(NB):
                for oc in range(NOC):
                    i = bb * NOC + oc
                    t = out_pool.tile([P, OC], dt.float32, tag="ot", name=f"ot{i}")
                    # two parallel copy chains: DVE and ACT
                    if i % 2 == 0:
                        nc.vector.tensor_copy(t[:], psum_b[i][:])
                        ring = nc.sync
                    else:
                        nc.scalar.activation(
                            t[:], psum_b[i][:], mybir.ActivationFunctionType.Identity
                        )
                        ring = nc.scalar
                    ring.dma_start(
                        out=outN[bb * P : (bb + 1) * P, oc * OC : (oc + 1) * OC],
                        in_=t[:],
                    )

    _hoist_preamble_work(nc, mybir)
    _split_excess_waits(nc, mybir)
    return nc


def kernel(
    x,
    embed_rows,
    embed_cols,
    embed_vals,
    w_rows,
    w_cols,
    w_vals,
    bias_idx,
    bias_vals,
):
    from concourse.bass_utils import run_bass_kernel_spmd

    # accept jax arrays (setup_inputs returns them) or numpy
    x = np.asarray(x)
    embed_rows = np.asarray(embed_rows)
    embed_cols = np.asarray(embed_cols)
    embed_vals = np.asarray(embed_vals)
    w_rows = np.asarray(w_rows)
    w_cols = np.asarray(w_cols)
    w_vals = np.asarray(w_vals)
    bias_idx = np.asarray(bias_idx)
    bias_vals = np.asarray(bias_vals)

    mode = os.environ.get("KERNEL_MODE", "bf16")

    # --- host-side weight prep (one-time, O(nnz)) --------------------------
    weT = (
        np.bincount(
            embed_cols.astype(np.int64) * R_EMB + embed_rows.astype(np.int64),
            weights=embed_vals.astype(np.float64),
            minlength=F_IN * R_EMB,
        )
        .reshape(F_IN, R_EMB)
        .astype(np.float32)
    )
    wT = (
        np.bincount(
            w_cols.astype(np.int64) * F_OUT + w_rows.astype(np.int64),
            weights=w_vals.astype(np.float64),
            minlength=F_MID * F_OUT,
        )
        .reshape(F_MID, F_OUT)
        .astype(np.float32)
    )
    bias = np.bincount(
        bias_idx.astype(np.int64), weights=bias_vals.astype(np.float64), minlength=F_OUT
    ).astype(np.float32)
    bias_row = np.ascontiguousarray(bias[None, :])
    ones_row = np.ones((1, P), np.float32)

    xT = np.ascontiguousarray(x.T.astype(np.float32))
    if mode == "bf16":
        import ml_dtypes

        np_dt = ml_dtypes.bfloat16
        xT = xT.astype(np_dt)
        weT = weT.astype(np_dt)
        wT = wT.astype(np_dt)

    key = ("nc", mode)
    if key not in _cache:
        _cache[key] = _build(mode)
    nc = _cache[key]

    in_maps = [
        {
            "xT": np.ascontiguousarray(xT[:, j * B_SH : (j + 1) * B_SH]),
            "weT": weT,
            "wT": wT,
            "bias_row": bias_row,
            "ones_row": ones_row,
        }
        for j in range(N_CORES)
    ]

    trace = bool(os.environ.get("KERNEL_TRACE"))
    kw = {}
    if trace:
        import concourse.bass_utils as bu

        bu.upload_artifacts = lambda t: t  # no artifact store in this container
        kw = dict(trace=True, tmpdir=os.environ.get("KERNEL_TRACE_DIR") or None)

    res = run_bass_kernel_spmd(nc, in_maps, list(range(N_CORES)), **kw)
    if trace:
        _cache["last_result"] = res

    out = np.empty((B, F_OUT), np.float32)
    for j in range(N_CORES):
        out[j * B_SH : (j + 1) * B_SH, :] = res.results[j]["outN"]
    return out


# revision 73
# speedup vs baseline: 1.0416x; 1.0416x over previous
"""ExpandingLinear (sparse EmbedLinear + sparse ExpandingLinear tail) on 8 trn2 cores.

Math:
    h  = relu(x @ W_e.T)          W_e sparse [R_EMB, F_IN]  (COO, 6.25% dense)
    x2 = concat([x, h], axis=1)
    y  = x2 @ W.T + bias          W   sparse [F_OUT, F_MID], bias sparse [F_OUT]

Strategy: densify the sparse weights on the host (one-time weight prep,
O(nnz) work), then run the O(nnz * B) compute as two dense matmuls on the
TensorEngine.  Data-parallel over the batch: each of the 8 cores gets
B/8 = 256 rows of x (as x.T columns) and the full dense weights.

Device schedule (per core):
    xt_sb  [128, 8*256]      x.T tiles resident in SBUF (f-major)
    we/wt  [128, G*1024] x k W_e.T / W.T row-block stripes streamed on the
                             sync HWDGE ring; PE chases the arrival front
    warm-up: ~3.4us of garbage matmuls so the HAM clock gate is at 2.4 GHz
    MM1: psum_h[r] += we[f,r].T @ xt[f]    -> relu -> hT[r] [128, 256]
    MM2: psum[b,oc] = ones.T @ bias_row    (K=1 bias broadcast, first)
                    += x2t[c,b].T @ wt[c,oc]  N=512, x2t = xt ++ hT tiles
    out: psum -> SBUF copies on DVE/ACT in parallel -> natural-layout
         [256, 1024] stores split across both HWDGE rings

Post-passes work around / trim framework overheads: _split_excess_waits
(walrus here caps sync waits at 1/instruction), _hoist_preamble_work
(start the DMA pipe during the preamble), and a leaner TileContext tail.

Modes (KERNEL_MODE env): "bf16" (default, ~42us, rel err ~1e-3) or "f32r"
(fp32 storage, full-rate fp32r matmuls, ~52us, rel err ~1.4e-4).
"""

import os

import numpy as np

B = 2048
F_IN = 1024
R_EMB = 1024
F_OUT = 1024
F_MID = F_IN + R_EMB
N_CORES = 8
B_SH = B // N_CORES  # 256

P = 128
NF = F_IN // P    # 8 f-tiles (MM1 contraction)
NR = R_EMB // P   # 8 r-tiles (MM1 outputs / psum tiles)
NC_T = F_MID // P  # 16 c-tiles (MM2 contraction)
NO = F_OUT // P   # 8 o-tiles (MM2 outputs)

_cache = {}


def _split_excess_waits(nc, mybir, max_waits=1):
    """Walrus in this container rejects instructions with >1 sync waits
    ("Too many sync wait commands").  Hoist excess waits onto same-engine
    NOPs placed immediately before the offending instruction."""
    cnt = 0
    for f in nc.m.functions:
        for b in f.blocks:
            out = []
            for inst in b.instructions:
                si = inst.sync_info
                if si is not None and len(si.on_wait) > max_waits:
                    waits = list(si.on_wait)
                    keep = waits[-max_waits:]
                    hoist = waits[:-max_waits]
                    for j in range(0, len(hoist), max_waits):
                        chunk = hoist[j : j + max_waits]
                        out.append(
                            mybir.InstNoOp(
                                name=f"{inst.name}_splitw{j}",
                                engine=inst.engine,
                                sync_info=mybir.SyncInfo(on_wait=chunk, on_update=[]),
                                bass_nofuse=True,
                            )
                        )
                        cnt += 1
                    inst.sync_info = mybir.SyncInfo(
                        on_wait=keep, on_update=list(si.on_update)
                    )
                out.append(inst)
            b.instructions = out
    return cnt


def _hoist_preamble_work(nc, mybir, max_dmas=2):
    """Move the first `max_dmas` wait-free SP load DMAs and the first
    wait-free Pool memset from the tile block into the main block, ahead
    of each engine's start-barrier EVSEM.  They only touch
    freshly-allocated SBUF, so running them during the preamble is safe;
    the DMA pipe starts ~3us earlier and the PE warm-up source is ready
    as soon as PE clears the barrier."""
    f = nc.m.functions[0]
    b0, b1 = f.blocks[0], f.blocks[1]
    moved_sp, moved_pool, rest = [], [], []
    sp_prefix = pool_prefix = True
    moved_pe = []
    pe_prefix = True
    for inst in b1.instructions:
        nm = type(inst).__name__
        si = inst.sync_info
        waits = bool(si and si.on_wait)
        if sp_prefix and inst.engine == mybir.EngineType.SP:
            if "DMA" in nm and not waits and len(moved_sp) < max_dmas:
                moved_sp.append(inst)
                continue
            sp_prefix = False
        if pool_prefix and inst.engine == mybir.EngineType.Pool:
            if nm == "InstMemset" and not waits:
                moved_pool.append(inst)
                pool_prefix = False
                continue
            pool_prefix = False
        if pe_prefix and inst.engine == mybir.EngineType.PE:
            # warm-up matmuls (they reference the wsrc tile): run them
            # during the start barrier so HAM is warm when real work begins;
            # their memset wait travels with them (memset is hoisted too)
            if nm in ("InstMatmult", "InstLdweights") and "wsrc" in inst.pretty_str():
                moved_pe.append(inst)
                continue
            pe_prefix = False
        rest.append(inst)
    il0 = list(b0.instructions)
    sp_pos = next(
        (i for i, inst in enumerate(il0) if inst.engine == mybir.EngineType.SP),
        len(il0),
    )
    il0 = il0[:sp_pos] + moved_sp + il0[sp_pos:]
    # warm-up memset goes to the very front of the Pool stream so the PE
    # warm-up can begin right after the engines' preamble table loads
    pool_pos = next(
        (i for i, inst in enumerate(il0) if inst.engine == mybir.EngineType.Pool),
        len(il0),
    )
    il0 = il0[:pool_pos] + moved_pool + il0[pool_pos:]
    pe_pos = next(
        (
            i
            for i, inst in enumerate(il0)
            if inst.engine == mybir.EngineType.PE
            and type(inst).__name__ == "InstDrain"
        ),
        len(il0),
    )
    il0 = il0[:pe_pos] + moved_pe + il0[pe_pos:]
    b0.instructions = il0
    b1.instructions = rest
    return len(moved_sp) + len(moved_pool) + len(moved_pe)


def _build(mode):
    import concourse.bass as bass
    import concourse.mybir as mybir
    import concourse.tile as tile

    # Leaner kernel tail: the stock _drain_and_barrier runs
    # drain -> barrier -> sem clears -> barrier.  The final barrier only
    # makes the other engines wait for SP's sem clears; execution ends when
    # every engine stream ends either way, so drop it.
    if not getattr(tile.TileContext, "_lean_tail", False):
        def _drain_and_barrier(self, tick_clock, wait_clock):
            from concourse.vector_clock import ScopedClock

            drain_inst = self.nc.sync.drain()
            wait_clock.add_sem_waits(
                drain_inst.ins, ScopedClock({None: tick_clock.global_clock})
            )
            self.nc.all_engine_barrier()
            assert self.sems is not None
            popped = self.nc._tile_sem_poison_stack.pop()
            assert popped is self._sem_poison
            self.nc.clear_and_free_semaphores(list(self.sems.allocated().values()))

        tile.TileContext._drain_and_barrier = _drain_and_barrier
        tile.TileContext._lean_tail = True

    dt = mybir.dt
    mm_dt = dt.float32r if mode == "f32r" else dt.bfloat16
    G1 = 1  # row-blocks per MM1 weight stripe (small: earliest PE start)
    G2 = 2  # row-blocks per MM2 weight stripe

    nc = bass.Bass("TRN2", target_bir_lowering=False, debug=False, num_devices=N_CORES)

    f32r = dt.float32r
    xT = nc.declare_dram_parameter("xT", [F_IN, B_SH], mm_dt, isOutput=False)
    weT = nc.declare_dram_parameter("weT", [F_IN, R_EMB], mm_dt, isOutput=False)
    wT = nc.declare_dram_parameter("wT", [F_MID, F_OUT], mm_dt, isOutput=False)
    bias_row = nc.declare_dram_parameter("bias_row", [1, F_OUT], f32r, isOutput=False)
    ones_row = nc.declare_dram_parameter("ones_row", [1, P], f32r, isOutput=False)
    outN = nc.declare_dram_parameter("outN", [B_SH, F_OUT], dt.float32, isOutput=True)

    with tile.TileContext(nc) as tc:
        with (
            tc.tile_pool(name="xt", bufs=1) as xt_pool,
            tc.tile_pool(name="w", bufs=8) as w_pool,
            tc.tile_pool(name="h", bufs=NR) as h_pool,
            tc.tile_pool(name="ot", bufs=NO) as out_pool,
            tc.tile_pool(name="bias", bufs=1) as bias_pool,
            tc.tile_pool(name="psum", bufs=8, space="PSUM") as psum_pool,
        ):
            # PE warm-up source: memset tile (always bf16 — warm-up only
            # exists to keep the HAM clock busy; values are irrelevant)
            wsrc = bias_pool.tile([P, B_SH], dt.bfloat16, name="wsrc")
            nc.gpsimd.memset(wsrc[:], 0)
            # dummy ACT op: pulls the activation LUT load into the preamble
            # window (internal queue) instead of lazily on the output path
            act_warm = bias_pool.tile([P, 2], dt.float32, name="act_warm")
            nc.gpsimd.memset(act_warm[:], 0)
            nc.scalar.activation(
                act_warm[:, 0:1], act_warm[:, 1:2],
                mybir.ActivationFunctionType.Identity,
            )

            # x.T on the sync ring, split in two so the first (hoisted)
            # chunk is small and MM1 f0..f3 become consumable earliest;
            # bias/ones on the scalar ring
            xt_sb = xt_pool.tile([P, NF * B_SH], mm_dt)
            HF = NF // 2
            nc.sync.dma_start(
                out=xt_sb[:, : HF * B_SH].rearrange("p (f b) -> p f b", b=B_SH),
                in_=xT[: HF * P, :].rearrange("(f p) b -> p f b", p=P),
            )
            bias_sb = bias_pool.tile([1, F_OUT], f32r, name="bias_sb")
            nc.scalar.dma_start(out=bias_sb[:], in_=bias_row[:])
            ones_sb = bias_pool.tile([1, P], f32r, name="ones_sb")
            nc.scalar.dma_start(out=ones_sb[:], in_=ones_row[:])

            # weight stripes on the sync ring only (a single HWDGE ring
            # sustains ~320 GB/s; splitting across two measured slower)
            we_t = []
            for k in range(NF // G1):
                t = w_pool.tile([P, G1 * R_EMB], mm_dt, tag="we", name=f"we{k}")
                nc.sync.dma_start(
                    out=t[:].rearrange("p (f r) -> p f r", r=R_EMB),
                    in_=weT[G1 * k * P : G1 * (k + 1) * P, :].rearrange(
                        "(f p) r -> p f r", p=P
                    ),
                )
                we_t.append(t)
                if k == HF - 1:
                    # second x.T half, ahead of the we stripes that need it
                    nc.sync.dma_start(
                        out=xt_sb[:, HF * B_SH :].rearrange(
                            "p (f b) -> p f b", b=B_SH
                        ),
                        in_=xT[HF * P :, :].rearrange("(f p) b -> p f b", p=P),
                    )
            wt_t = []
            for k in range(NC_T // G2):
                t = w_pool.tile([P, G2 * F_OUT], mm_dt, tag="wt", name=f"wt{k}")
                nc.sync.dma_start(
                    out=t[:].rearrange("p (c o) -> p c o", o=F_OUT),
                    in_=wT[G2 * k * P : G2 * (k + 1) * P, :].rearrange(
                        "(c p) o -> p c o", p=P
                    ),
                )
                wt_t.append(t)

            # MM1: psum_h[r] = sum_f we[f, r-block].T @ xt[f]
            psum_h = [
                psum_pool.tile([P, B_SH], dt.float32, tag="acc", name=f"ph{r}")
                for r in range(NR)
            ]

            # PE warm-up: ~4us of garbage matmuls starting right after the
            # memset so the HAM clock gate is at 2.4 GHz when real matmuls
            # start.  Results land in psum_h[0] partitions 0-7 and are wiped
            # by MM1's start=True.
            for _ in range(16):
                nc.tensor.matmul(
                    out=psum_h[0][0:8, :],
                    lhsT=wsrc[:, 0:8],
                    rhs=wsrc[:],
                    start=True,
                    stop=True,
                )
            for f in range(NF):
                lhs_base = we_t[f // G1]
                off = (f % G1) * R_EMB
                rhs = xt_sb[:, f * B_SH : (f + 1) * B_SH]
                for r in range(NR):
                    nc.tensor.matmul(
                        out=psum_h[r][:],
                        lhsT=lhs_base[:, off + r * P : off + (r + 1) * P],
                        rhs=rhs,
                        start=(f == 0),
                        stop=(f == NF - 1),
                    )

            hT = []
            for r in range(NR):
                t = h_pool.tile([P, B_SH], mm_dt, tag="h", name=f"h{r}")
                nc.vector.tensor_scalar_max(t[:], psum_h[r][:], 0.0)
                hT.append(t)

            # MM2 (activations stationary, weights moving, N=512):
            #   out[b-block, o-chunk] = sum_c x2t[c, b-block].T @ wt[c, o-chunk]
            # x2t tiles are the resident xt slices (c<8) and hT tiles (c>=8).
            NB = 2   # b-blocks of 128
            NOC = 2  # o-chunks of 512
            OC = F_OUT // NOC
            psum_b = [
                psum_pool.tile([P, OC], dt.float32, tag="acc", name=f"pb{i}")
                for i in range(NB * NOC)
            ]
            # bias first via K=1 matmul: psum = ones.T @ bias_row (broadcast
            # over b); keeps the bias add off the kernel tail
            for bb in range(NB):
                for oc in range(NOC):
                    nc.tensor.matmul(
                        out=psum_b[bb * NOC + oc][:],
                        lhsT=ones_sb[:],
                        rhs=bias_sb[:, oc * OC : (oc + 1) * OC],
                        start=True,
                        stop=False,
                    )
            for c in range(NC_T):
                rhs_base = wt_t[c // G2]
                off = (c % G2) * F_OUT
                for bb in range(NB):
                    if c < NF:
                        lhsT = xt_sb[:, c * B_SH + bb * P : c * B_SH + (bb + 1) * P]
                    else:
                        lhsT = hT[c - NF][:, bb * P : (bb + 1) * P]
                    for oc in range(NOC):
                        nc.tensor.matmul(
                            out=psum_b[bb * NOC + oc][:],
                            lhsT=lhsT,
                            rhs=rhs_base[:, off + oc * OC : off + (oc + 1) * OC],
                            start=False,
                            stop=(c == NC_T - 1),
                        )

            for bb in range(NB):
                for oc in range(NOC):
                    i = bb * NOC + oc
                    t = out_pool.tile([P, OC], dt.float32, tag="ot", name=f"ot{i}")
                    # two parallel copy chains: DVE and ACT
                    if i % 2 == 0:
                        nc.vector.tensor_copy(t[:], psum_b[i][:])
                        ring = nc.sync
                    else:
                        nc.scalar.activation(
                            t[:], psum_b[i][:], mybir.ActivationFunctionType.Identity
                        )
                        ring = nc.scalar
                    ring.dma_start(
                        out=outN[bb * P : (bb + 1) * P, oc * OC : (oc + 1) * OC],
                        in_=t[:],
                    )

    _hoist_preamble_work(nc, mybir)
    _split_excess_waits(nc, mybir)
    return nc


def kernel(
    x,
    embed_rows,
    embed_cols,
    embed_vals,
    w_rows,
    w_cols,
    w_vals,
    bias_idx,
    bias_vals,
):
    from concourse.bass_utils import run_bass_kernel_spmd

    # accept jax arrays (setup_inputs returns them) or numpy
    x = np.asarray(x)
    embed_rows = np.asarray(embed_rows)
    embed_cols = np.asarray(embed_cols)
    embed_vals = np.asarray(embed_vals)
    w_rows = np.asarray(w_rows)
    w_cols = np.asarray(w_cols)
    w_vals = np.asarray(w_vals)
    bias_idx = np.asarray(bias_idx)
    bias_vals = np.asarray(bias_vals)

    mode = os.environ.get("KERNEL_MODE", "bf16")

    # --- host-side weight prep (one-time, O(nnz)) --------------------------
    weT = (
        np.bincount(
            embed_cols.astype(np.int64) * R_EMB + embed_rows.astype(np.int64),
            weights=embed_vals.astype(np.float64),
            minlength=F_IN * R_EMB,
        )
        .reshape(F_IN, R_EMB)
        .astype(np.float32)
    )
    wT = (
        np.bincount(
            w_cols.astype(np.int64) * F_OUT + w_rows.astype(np.int64),
            weights=w_vals.astype(np.float64),
            minlength=F_MID * F_OUT,
        )
        .reshape(F_MID, F_OUT)
        .astype(np.float32)
    )
    bias = np.bincount(
        bias_idx.astype(np.int64), weights=bias_vals.astype(np.float64), minlength=F_OUT
    ).astype(np.float32)
    bias_row = np.ascontiguousarray(bias[None, :])
    ones_row = np.ones((1, P), np.float32)

    xT = np.ascontiguousarray(x.T.astype(np.float32))
    if mode == "bf16":
        import ml_dtypes

        np_dt = ml_dtypes.bfloat16
        xT = xT.astype(np_dt)
        weT = weT.astype(np_dt)
        wT = wT.astype(np_dt)

    key = ("nc", mode)
    if key not in _cache:
        _cache[key] = _build(mode)
    nc = _cache[key]

    in_maps = [
        {
            "xT": np.ascontiguousarray(xT[:, j * B_SH : (j + 1) * B_SH]),
            "weT": weT,
            "wT": wT,
            "bias_row": bias_row,
            "ones_row": ones_row,
        }
        for j in range(N_CORES)
    ]

    trace = bool(os.environ.get("KERNEL_TRACE"))
    kw = {}
    if trace:
        import concourse.bass_utils as bu

        bu.upload_artifacts = lambda t: t  # no artifact store in this container
        kw = dict(trace=True, tmpdir=os.environ.get("KERNEL_TRACE_DIR") or None)

    res = run_bass_kernel_spmd(nc, in_maps, list(range(N_CORES)), **kw)
    if trace:
        _cache["last_result"] = res

    out = np.empty((B, F_OUT), np.float32)
    for j in range(N_CORES):
        out[j * B_SH : (j + 1) * B_SH, :] = res.results[j]["outN"]
    return out


# revision 74
# speedup vs baseline: 1.0565x; 1.0143x over previous
"""ExpandingLinear (sparse EmbedLinear + sparse ExpandingLinear tail) on 8 trn2 cores.

Math:
    h  = relu(x @ W_e.T)          W_e sparse [R_EMB, F_IN]  (COO, 6.25% dense)
    x2 = concat([x, h], axis=1)
    y  = x2 @ W.T + bias          W   sparse [F_OUT, F_MID], bias sparse [F_OUT]

Strategy: densify the sparse weights on the host (one-time weight prep,
O(nnz) work), then run the O(nnz * B) compute as two dense matmuls on the
TensorEngine.  Data-parallel over the batch: each of the 8 cores gets
B/8 = 256 rows of x (as x.T columns) and the full dense weights.

Device schedule (per core):
    xt_sb  [128, 8*256]      x.T tiles resident in SBUF (f-major)
    we/wt  [128, G*1024] x k W_e.T / W.T row-block stripes streamed on the
                             sync HWDGE ring; PE chases the arrival front
    warm-up: ~3.4us of garbage matmuls so the HAM clock gate is at 2.4 GHz
    MM1: psum_h[r] += we[f,r].T @ xt[f]    -> relu -> hT[r] [128, 256]
    MM2: psum[b,oc] = ones.T @ bias_row    (K=1 bias broadcast, first)
                    += x2t[c,b].T @ wt[c,oc]  N=512, x2t = xt ++ hT tiles
    out: psum -> SBUF copies on DVE/ACT in parallel -> natural-layout
         [256, 1024] stores split across both HWDGE rings

Post-passes work around / trim framework overheads: _split_excess_waits
(walrus here caps sync waits at 1/instruction), _hoist_preamble_work
(start the DMA pipe during the preamble), and a leaner TileContext tail.

Modes (KERNEL_MODE env): "bf16" (default, ~42us, rel err ~1e-3) or "f32r"
(fp32 storage, full-rate fp32r matmuls, ~52us, rel err ~1.4e-4).
"""

import os

import numpy as np

B = 2048
F_IN = 1024
R_EMB = 1024
F_OUT = 1024
F_MID = F_IN + R_EMB
N_CORES = 8
B_SH = B // N_CORES  # 256

P = 128
NF = F_IN // P    # 8 f-tiles (MM1 contraction)
NR = R_EMB // P   # 8 r-tiles (MM1 outputs / psum tiles)
NC_T = F_MID // P  # 16 c-tiles (MM2 contraction)
NO = F_OUT // P   # 8 o-tiles (MM2 outputs)

_cache = {}


def _split_excess_waits(nc, mybir, max_waits=1):
    """Walrus in this container rejects instructions with >1 sync waits
    ("Too many sync wait commands").  Hoist excess waits onto same-engine
    NOPs placed immediately before the offending instruction."""
    cnt = 0
    for f in nc.m.functions:
        for b in f.blocks:
            out = []
            for inst in b.instructions:
                si = inst.sync_info
                if si is not None and len(si.on_wait) > max_waits:
                    waits = list(si.on_wait)
                    keep = waits[-max_waits:]
                    hoist = waits[:-max_waits]
                    for j in range(0, len(hoist), max_waits):
                        chunk = hoist[j : j + max_waits]
                        out.append(
                            mybir.InstNoOp(
                                name=f"{inst.name}_splitw{j}",
                                engine=inst.engine,
                                sync_info=mybir.SyncInfo(on_wait=chunk, on_update=[]),
                                bass_nofuse=True,
                            )
                        )
                        cnt += 1
                    inst.sync_info = mybir.SyncInfo(
                        on_wait=keep, on_update=list(si.on_update)
                    )
                out.append(inst)
            b.instructions = out
    return cnt


def _hoist_preamble_work(nc, mybir, max_dmas=2):
    """Move the first `max_dmas` wait-free SP load DMAs and the first
    wait-free Pool memset from the tile block into the main block, ahead
    of each engine's start-barrier EVSEM.  They only touch
    freshly-allocated SBUF, so running them during the preamble is safe;
    the DMA pipe starts ~3us earlier and the PE warm-up source is ready
    as soon as PE clears the barrier."""
    f = nc.m.functions[0]
    b0, b1 = f.blocks[0], f.blocks[1]
    moved_sp, moved_pool, rest = [], [], []
    sp_prefix = pool_prefix = True
    moved_pe = []
    pe_prefix = True
    for inst in b1.instructions:
        nm = type(inst).__name__
        si = inst.sync_info
        waits = bool(si and si.on_wait)
        if sp_prefix and inst.engine == mybir.EngineType.SP:
            if "DMA" in nm and not waits and len(moved_sp) < max_dmas:
                moved_sp.append(inst)
                continue
            sp_prefix = False
        if pool_prefix and inst.engine == mybir.EngineType.Pool:
            if nm == "InstMemset" and not waits:
                moved_pool.append(inst)
                pool_prefix = False
                continue
            pool_prefix = False
        if pe_prefix and inst.engine == mybir.EngineType.PE:
            # warm-up matmuls (they reference the wsrc tile): run them
            # during the start barrier so HAM is warm when real work begins;
            # their memset wait travels with them (memset is hoisted too)
            if nm in ("InstMatmult", "InstLdweights") and "wsrc" in inst.pretty_str():
                moved_pe.append(inst)
                continue
            pe_prefix = False
        rest.append(inst)
    il0 = list(b0.instructions)
    sp_pos = next(
        (i for i, inst in enumerate(il0) if inst.engine == mybir.EngineType.SP),
        len(il0),
    )
    il0 = il0[:sp_pos] + moved_sp + il0[sp_pos:]
    pool_pos = next(
        (
            i
            for i, inst in enumerate(il0)
            if inst.engine == mybir.EngineType.Pool
            and type(inst).__name__ == "InstDrain"
        ),
        len(il0),
    )
    il0 = il0[:pool_pos] + moved_pool + il0[pool_pos:]
    pe_pos = next(
        (
            i
            for i, inst in enumerate(il0)
            if inst.engine == mybir.EngineType.PE
            and type(inst).__name__ == "InstDrain"
        ),
        len(il0),
    )
    il0 = il0[:pe_pos] + moved_pe + il0[pe_pos:]
    b0.instructions = il0
    b1.instructions = rest
    return len(moved_sp) + len(moved_pool) + len(moved_pe)


def _build(mode):
    import concourse.bass as bass
    import concourse.mybir as mybir
    import concourse.tile as tile

    # Leaner kernel tail: the stock _drain_and_barrier runs
    # drain -> barrier -> sem clears -> barrier.  The final barrier only
    # makes the other engines wait for SP's sem clears; execution ends when
    # every engine stream ends either way, so drop it.
    if not getattr(tile.TileContext, "_lean_tail", False):
        def _drain_and_barrier(self, tick_clock, wait_clock):
            from concourse.vector_clock import ScopedClock

            drain_inst = self.nc.sync.drain()
            wait_clock.add_sem_waits(
                drain_inst.ins, ScopedClock({None: tick_clock.global_clock})
            )
            self.nc.all_engine_barrier()
            assert self.sems is not None
            popped = self.nc._tile_sem_poison_stack.pop()
            assert popped is self._sem_poison
            self.nc.clear_and_free_semaphores(list(self.sems.allocated().values()))

        tile.TileContext._drain_and_barrier = _drain_and_barrier
        tile.TileContext._lean_tail = True

    dt = mybir.dt
    mm_dt = dt.float32r if mode == "f32r" else dt.bfloat16
    G1 = 1  # row-blocks per MM1 weight stripe (small: earliest PE start)
    G2 = 2  # row-blocks per MM2 weight stripe

    nc = bass.Bass("TRN2", target_bir_lowering=False, debug=False, num_devices=N_CORES)

    f32r = dt.float32r
    xT = nc.declare_dram_parameter("xT", [F_IN, B_SH], mm_dt, isOutput=False)
    weT = nc.declare_dram_parameter("weT", [F_IN, R_EMB], mm_dt, isOutput=False)
    wT = nc.declare_dram_parameter("wT", [F_MID, F_OUT], mm_dt, isOutput=False)
    bias_row = nc.declare_dram_parameter("bias_row", [1, F_OUT], f32r, isOutput=False)
    ones_row = nc.declare_dram_parameter("ones_row", [1, P], f32r, isOutput=False)
    outN = nc.declare_dram_parameter("outN", [B_SH, F_OUT], dt.float32, isOutput=True)

    with tile.TileContext(nc) as tc:
        with (
            tc.tile_pool(name="xt", bufs=1) as xt_pool,
            tc.tile_pool(name="w", bufs=8) as w_pool,
            tc.tile_pool(name="h", bufs=NR) as h_pool,
            tc.tile_pool(name="ot", bufs=NO) as out_pool,
            tc.tile_pool(name="bias", bufs=1) as bias_pool,
            tc.tile_pool(name="psum", bufs=8, space="PSUM") as psum_pool,
        ):
            # PE warm-up source: memset tile (always bf16 — warm-up only
            # exists to keep the HAM clock busy; values are irrelevant)
            wsrc = bias_pool.tile([P, B_SH], dt.bfloat16, name="wsrc")
            nc.gpsimd.memset(wsrc[:], 0)
            # dummy ACT op: pulls the activation LUT load into the preamble
            # window (internal queue) instead of lazily on the output path
            act_warm = bias_pool.tile([P, 2], dt.float32, name="act_warm")
            nc.gpsimd.memset(act_warm[:], 0)
            nc.scalar.activation(
                act_warm[:, 0:1], act_warm[:, 1:2],
                mybir.ActivationFunctionType.Identity,
            )

            # x.T first on the sync ring (gates all of MM1); bias on scalar
            xt_sb = xt_pool.tile([P, NF * B_SH], mm_dt)
            nc.sync.dma_start(
                out=xt_sb[:].rearrange("p (f b) -> p f b", b=B_SH),
                in_=xT[:].rearrange("(f p) b -> p f b", p=P),
            )
            bias_sb = bias_pool.tile([1, F_OUT], f32r, name="bias_sb")
            nc.scalar.dma_start(out=bias_sb[:], in_=bias_row[:])
            ones_sb = bias_pool.tile([1, P], f32r, name="ones_sb")
            nc.scalar.dma_start(out=ones_sb[:], in_=ones_row[:])

            # weight stripes on the sync ring only (a single HWDGE ring
            # sustains ~320 GB/s; splitting across two measured slower)
            we_t = []
            for k in range(NF // G1):
                t = w_pool.tile([P, G1 * R_EMB], mm_dt, tag="we", name=f"we{k}")
                nc.sync.dma_start(
                    out=t[:].rearrange("p (f r) -> p f r", r=R_EMB),
                    in_=weT[G1 * k * P : G1 * (k + 1) * P, :].rearrange(
                        "(f p) r -> p f r", p=P
                    ),
                )
                we_t.append(t)
            wt_t = []
            for k in range(NC_T // G2):
                t = w_pool.tile([P, G2 * F_OUT], mm_dt, tag="wt", name=f"wt{k}")
                nc.sync.dma_start(
                    out=t[:].rearrange("p (c o) -> p c o", o=F_OUT),
                    in_=wT[G2 * k * P : G2 * (k + 1) * P, :].rearrange(
                        "(c p) o -> p c o", p=P
                    ),
                )
                wt_t.append(t)

            # MM1: psum_h[r] = sum_f we[f, r-block].T @ xt[f]
            psum_h = [
                psum_pool.tile([P, B_SH], dt.float32, tag="acc", name=f"ph{r}")
                for r in range(NR)
            ]

            # PE warm-up: ~4us of garbage matmuls starting right after the
            # memset so the HAM clock gate is at 2.4 GHz when real matmuls
            # start.  Results land in psum_h[0] partitions 0-7 and are wiped
            # by MM1's start=True.
            for _ in range(16):
                nc.tensor.matmul(
                    out=psum_h[0][0:8, :],
                    lhsT=wsrc[:, 0:8],
                    rhs=wsrc[:],
                    start=True,
                    stop=True,
                )
            for f in range(NF):
                lhs_base = we_t[f // G1]
                off = (f % G1) * R_EMB
                rhs = xt_sb[:, f * B_SH : (f + 1) * B_SH]
                for r in range(NR):
                    nc.tensor.matmul(
                        out=psum_h[r][:],
                        lhsT=lhs_base[:, off + r * P : off + (r + 1) * P],
                        rhs=rhs,
                        start=(f == 0),
                        stop=(f == NF - 1),
                    )

            hT = []
            for r in range(NR):
                t = h_pool.tile([P, B_SH], mm_dt, tag="h", name=f"h{r}")
                nc.vector.tensor_scalar_max(t[:], psum_h[r][:], 0.0)
                hT.append(t)

            # MM2 (activations stationary, weights moving, N=512):
            #   out[b-block, o-chunk] = sum_c x2t[c, b-block].T @ wt[c, o-chunk]
            # x2t tiles are the resident xt slices (c<8) and hT tiles (c>=8).
            NB = 2   # b-blocks of 128
            NOC = 2  # o-chunks of 512
            OC = F_OUT // NOC
            psum_b = [
                psum_pool.tile([P, OC], dt.float32, tag="acc", name=f"pb{i}")
                for i in range(NB * NOC)
            ]
            # bias first via K=1 matmul: psum = ones.T @ bias_row (broadcast
            # over b); keeps the bias add off the kernel tail
            for bb in range(NB):
                for oc in range(NOC):
                    nc.tensor.matmul(
                        out=psum_b[bb * NOC + oc][:],
                        lhsT=ones_sb[:],
                        rhs=bias_sb[:, oc * OC : (oc + 1) * OC],
                        start=True,
                        stop=False,
                    )
            for c in range(NC_T):
                rhs_base = wt_t[c // G2]
                off = (c % G2) * F_OUT
                for bb in range(NB):
                    if c < NF:
                        lhsT = xt_sb[:, c * B_SH + bb * P : c * B_SH + (bb + 1) * P]
                    else:
                        lhsT = hT[c - NF][:, bb * P : (bb + 1) * P]
                    for oc in range(NOC):
                        nc.tensor.matmul(
                            out=psum_b[bb * NOC + oc][:],
                            lhsT=lhsT,
                            rhs=rhs_base[:, off + oc * OC : off + (oc + 1) * OC],
                            start=False,
                            stop=(c == NC_T - 1),
                        )

            for bb in range(NB):
                for oc in range(NOC):
                    i = bb * NOC + oc
                    t = out_pool.tile([P, OC], dt.float32, tag="ot", name=f"ot{i}")
                    # two parallel copy chains: DVE and ACT
                    if i % 2 == 0:
                        nc.vector.tensor_copy(t[:], psum_b[i][:])
                        ring = nc.sync
                    else:
                        nc.scalar.activation(
                            t[:], psum_b[i][:], mybir.ActivationFunctionType.Identity
                        )
                        ring = nc.scalar
                    ring.dma_start(
                        out=outN[bb * P : (bb + 1) * P, oc * OC : (oc + 1) * OC],
                        in_=t[:],
                    )

    _hoist_preamble_work(nc, mybir)
    _split_excess_waits(nc, mybir)
    return nc


def kernel(
    x,
    embed_rows,
    embed_cols,
    embed_vals,
    w_rows,
    w_cols,
    w_vals,
    bias_idx,
    bias_vals,
):
    from concourse.bass_utils import run_bass_kernel_spmd

    # accept jax arrays (setup_inputs returns them) or numpy
    x = np.asarray(x)
    embed_rows = np.asarray(embed_rows)
    embed_cols = np.asarray(embed_cols)
    embed_vals = np.asarray(embed_vals)
    w_rows = np.asarray(w_rows)
    w_cols = np.asarray(w_cols)
    w_vals = np.asarray(w_vals)
    bias_idx = np.asarray(bias_idx)
    bias_vals = np.asarray(bias_vals)

    mode = os.environ.get("KERNEL_MODE", "bf16")

    # --- host-side weight prep (one-time, O(nnz)) --------------------------
    weT = (
        np.bincount(
            embed_cols.astype(np.int64) * R_EMB + embed_rows.astype(np.int64),
            weights=embed_vals.astype(np.float64),
            minlength=F_IN * R_EMB,
        )
        .reshape(F_IN, R_EMB)
        .astype(np.float32)
    )
    wT = (
        np.bincount(
            w_cols.astype(np.int64) * F_OUT + w_rows.astype(np.int64),
            weights=w_vals.astype(np.float64),
            minlength=F_MID * F_OUT,
        )
        .reshape(F_MID, F_OUT)
        .astype(np.float32)
    )
    bias = np.bincount(
        bias_idx.astype(np.int64), weights=bias_vals.astype(np.float64), minlength=F_OUT
    ).astype(np.float32)
    bias_row = np.ascontiguousarray(bias[None, :])
    ones_row = np.ones((1, P), np.float32)

    xT = np.ascontiguousarray(x.T.astype(np.float32))
    if mode == "bf16":
        import ml_dtypes

        np_dt = ml_dtypes.bfloat16
        xT = xT.astype(np_dt)
        weT = weT.astype(np_dt)
        wT = wT.astype(np_dt)

    key = ("nc", mode)
    if key not in _cache:
        _cache[key] = _build(mode)
    nc = _cache[key]

    in_maps = [
        {
            "xT": np.ascontiguousarray(xT[:, j * B_SH : (j + 1) * B_SH]),
            "weT": weT,
            "wT": wT,
            "bias_row": bias_row,
            "ones_row": ones_row,
        }
        for j in range(N_CORES)
    ]

    trace = bool(os.environ.get("KERNEL_TRACE"))
    kw = {}
    if trace:
        import concourse.bass_utils as bu

        bu.upload_artifacts = lambda t: t  # no artifact store in this container
        kw = dict(trace=True, tmpdir=os.environ.get("KERNEL_TRACE_DIR") or None)

    res = run_bass_kernel_spmd(nc, in_maps, list(range(N_CORES)), **kw)
    if trace:
        _cache["last_result"] = res

    out = np.empty((B, F_OUT), np.float32)
    for j in range(N_CORES):
        out[j * B_SH : (j + 1) * B_SH, :] = res.results[j]["outN"]
    return out
